# revision 37
# baseline (speedup 1.0000x reference)
"""Trainium2 Bass kernel for nn_BaselineTargetHead (per-sample dynamic MLP).

Strategy: data-parallel over 8 NeuronCores, 8 samples per core.
Per sample the chain is 5 per-sample linear layers over 64 spatial positions:
  [1024,2048] @ [2048,64] -> sigmoid -> ... -> [1,128] @ [128,64] + b

fc1-fc4 weights (99.9% of bytes) and the input x ship as fp8 e3m4 (4
mantissa bits). Host pre-scales weights by 64 (x by 2) to center N(0,0.02)
data in e3m4's normal range; the inverse scale folds into the ScalarE
activation's `scale`. fc5 weights stay fp16: the output is a 128-term dot
product with no downstream averaging, so fc5 quantization dominates the
error budget (quantizing w5 alone costs 1.4e-2 rel err; w1-w4 cost ~1e-3).

The kernel sits at the ridge: Tensor ~55-61 us busy (40-47 ns per
LDWEIGHTS+MATMUL pair, 171 pairs/sample) vs DMA ~68 us on a single queue.
The 8 cores contend for chip HBM (~2.6-2.9 TB/s aggregate), so per-core
stream rate tops out around 330-350 B/ns; splitting across two HWDGE
queues (or gpsimd SWDGE) derates the shared fabric and loses. Scheduling
details that matter:
  - everything lives in SBUF simultaneously (~186 KB/partition), so all
    DMAs are issued upfront with no tile rotation. Bulk DMAs must NOT be
    issued from the ACT engine: the tile framework's per-queue flow
    control would block ScalarE (and all activations) behind transfers.
  - weight DMAs are typed fp16 and bitcast to fp8 at the matmul.
  - fc1 is laid out m-major (col = m*2048 + k*128) and shipped in chunks
    that enable complete m-tiles immediately; sample 0's fc1 arrives in
    four chunks so the PE starts early.
  - the LAST sample's fc1 m0-3 chunks are spread early in the stream
    (after s4/s5) and their matmuls emitted inside blocks s5/s6, filling
    the mid-stream DMA-paced PE idle; only half its fc1 plus fc2-5
    remains after the final stream byte.
  - the previous sample's tiny fc4/fc5 are interleaved into fc1's m-groups
    so their input activations (314 ns ScalarE latency each) resolve
    behind ~2.7 us of fc1 matmuls instead of stalling the PE (~1.4
    us/sample of layer-boundary gaps otherwise).
  - matmul: lhsT = W^T tile [128(Cin), 128(Cout)] fp8 (FWL halves the
    weight-load time), rhs = activation tile [128(Cin), 64(spatial)] fp16,
    accumulated over Cin tiles in PSUM fp32. ScalarE applies
    scale+bias+sigmoid fused, writing fp16 tiles that feed the next layer
    without any transposition.
"""

import numpy as np
import ml_dtypes

import concourse.bass as bass
import concourse.mybir as mybir
import concourse.tile as tile
from concourse.bass_utils import run_bass_kernel_spmd

N_CORES = 8
B = 64
S_PER_CORE = B // N_CORES  # 8 samples per core
HW = 64  # 8x8 spatial positions
LAYERS = [(2048, 1024), (1024, 512), (512, 256), (256, 128)]  # (Cin, Cout) of fc1..fc4
W_SCALE_FP8 = 64.0  # host multiplies fp8 weights by this; kernel divides back
X_SCALE_FP8 = 2.0  # same for the input x image
A_COLS = (LAYERS[0][0] // 128) * LAYERS[0][1]  # 16384 fp8 cols (fc1, m-major)
B_COLS = sum((ci // 128) * co for ci, co in LAYERS[1:])  # 5376 fp8 cols (fc2-4)
X_COLS = (2048 // 128) * HW  # 1024
W5_COLS = 32  # w5 zero-padded to 32 cols for a legal M=32 matmul
# bias image columns per sample: fc1 m0..7 | fc2 m0..3 | fc3 m0..1 | fc4 m0 | fc5
BIAS_COL0 = [0, 8, 12, 14]
BIAS_COLS = 16
# per-layer PSUM scale to undo the host-side fp8 pre-scaling
ACT_SCALE = [
    1.0 / (W_SCALE_FP8 * X_SCALE_FP8),
    1.0 / W_SCALE_FP8,
    1.0 / W_SCALE_FP8,
    1.0 / W_SCALE_FP8,
]
# per-layer base fp8 column of each layer's weights within the fc2-4 block
B_OFF = [0, 4096, 4096 + 1024]  # fc2, fc3, fc4


def _split_ctrl_multiwaits(nc):
    """walrus in this env rejects >1 sync-wait per instruction. Move extra
    waits onto NOPs placed immediately before, on the same engine — engines
    execute in order, so this is semantically identical."""
    n_fixed = 0
    for bb in nc.main_func.blocks:
        insts = bb.instructions
        i = 0
        while i < len(insts):
            ins = insts[i]
            si = ins.sync_info
            if si is not None and si.on_wait and len(si.on_wait) > 1:
                waits = list(si.on_wait)
                new_nops = []
                for j, w in enumerate(waits[1:]):
                    nop = mybir.InstNoOp(name=f"{ins.name}-splitw-{j}", ins=[], outs=[])
                    nop.engine = ins.engine
                    nop.sync_info = mybir.SyncInfo(on_update=[], on_wait=[w])
                    new_nops.append(nop)
                si.on_wait = [waits[0]]
                insts[i:i] = new_nops
                i += len(new_nops)
                n_fixed += 1
            i += 1
    return n_fixed


def _build_nc():
    f8 = mybir.dt.float8e3
    f16 = mybir.dt.float16
    f32 = mybir.dt.float32
    nc = bass.Bass()
    # weight/x images carry fp8 bytes but are typed fp16 for the DMA;
    # compute slices bitcast back to fp8.
    wslab_d = nc.dram_tensor(
        "wslab", [S_PER_CORE, 128, (A_COLS + B_COLS) // 2], f16, kind="ExternalInput"
    )
    ximg_d = nc.dram_tensor(
        "ximg", [128, S_PER_CORE * X_COLS // 2], f16, kind="ExternalInput"
    )
    w5img_d = nc.dram_tensor("w5img", [128, S_PER_CORE * W5_COLS], f16, kind="ExternalInput")
    bias_d = nc.dram_tensor("bias", [128, S_PER_CORE * BIAS_COLS], f32, kind="ExternalInput")
    out_d = nc.dram_tensor("out", [1, S_PER_CORE * HW], f32, kind="ExternalOutput")

    sig = mybir.ActivationFunctionType.Sigmoid
    ident = mybir.ActivationFunctionType.Identity

    with tile.TileContext(nc) as tc:
        with (
            tc.tile_pool(name="wpool", bufs=1) as wpool,
            tc.tile_pool(name="qpool", bufs=2) as qpool,
            tc.tile_pool(name="psum", bufs=6, space="PSUM") as psum_pool,
        ):
            # ---- all DMAs issued upfront; everything fits in SBUF ----
            # SP queue: sample 0's x slice first (it gates the first matmul),
            # then the weight slabs, sample-major: fc1 m0-3 | fc1 m4-7 | fc2-4
            # (sample 0's fc1 in four chunks so the PE starts sooner).
            # x rides the ACT queue entirely (sample 0's slice first, it
            # gates the first matmul) so the weight stream starts 0.4 us
            # earlier and its end shifts the same amount.
            x_sb = wpool.tile([128, S_PER_CORE * X_COLS // 2], f16)
            nc.scalar.dma_start(x_sb[:, 0 : X_COLS // 2], ximg_d[:, 0 : X_COLS // 2])
            wa_sb, wb_sb = [], []
            for s in range(S_PER_CORE):
                ta = wpool.tile([128, A_COLS // 2], f16, name=f"wa{s}")
                wa_sb.append(ta)
                tb = wpool.tile([128, B_COLS // 2], f16, name=f"wb{s}")
                wb_sb.append(tb)
            AH = A_COLS // 2  # 8192 f16 cols of fc1 per sample
            QP = AH // 4  # 2048 f16 cols per m-pair (m-major layout)

            def issue_a(s, a, b):
                nc.sync.dma_start(wa_sb[s][:, a:b], wslab_d[s, :, a:b])

            def issue_b(s):
                nc.sync.dma_start(
                    wb_sb[s][:], wslab_d[s, :, AH : AH + B_COLS // 2]
                )

            LS = S_PER_CORE - 1
            # Sample-major stream, but the LAST sample's fc1 m-pairs are
            # spread early (after s4/s5) so its matmuls can run in the
            # mid-stream DMA-paced idle and the serial tail shrinks to
            # ~half of fc1 + fc2-5.
            for s in range(LS):
                nchunk = 4 if s == 0 else 2
                step = AH // nchunk
                for j in range(nchunk):
                    issue_a(s, j * step, (j + 1) * step)
                issue_b(s)
                if s == LS - 3:
                    issue_a(LS, 0, QP)  # s7 fc1 m0-1
                if s == LS - 2:
                    issue_a(LS, QP, 2 * QP)  # s7 fc1 m2-3
            issue_a(LS, 2 * QP, 3 * QP)  # s7 fc1 m4-5
            issue_a(LS, 3 * QP, 4 * QP)  # s7 fc1 m6-7
            issue_b(LS)
            # ACT queue: rest of x, bias, w5 — small transfers ride the
            # second queue essentially for free; it drains early so the
            # per-sample output DMAs are never stuck behind weight traffic.
            nc.scalar.dma_start(x_sb[:, X_COLS // 2 :], ximg_d[:, X_COLS // 2 :])
            bias_sb = wpool.tile([128, S_PER_CORE * BIAS_COLS], f32)
            nc.scalar.dma_start(bias_sb[:], bias_d[:])
            w5_sb = wpool.tile([128, S_PER_CORE * W5_COLS], f16)
            nc.scalar.dma_start(w5_sb[:], w5img_d[:])
            # all samples' outputs land in partition 0 of one tile
            # (sample s -> columns s*HW..(s+1)*HW)
            ot_all = wpool.tile([128, S_PER_CORE * HW], f32)

            # ---- compute ----
            def w_slice(s, li, k, m):
                """fp8 lhsT [128, 128] for (layer, k-tile, m-tile) of sample s."""
                if li == 0:
                    col = m * 2048 + k * 128  # m-major fc1 layout
                    return wa_sb[s][:, col // 2 : col // 2 + 64].bitcast(
                        mybir.dt.float8e3
                    )
                col = B_OFF[li - 1] + k * LAYERS[li][1] + m * 128
                return wb_sb[s][:, col // 2 : col // 2 + 64].bitcast(mybir.dt.float8e3)

            def x_slice(s, k):
                c = s * X_COLS + k * HW
                return x_sb[:, c // 2 : c // 2 + HW // 2].bitcast(mybir.dt.float8e3)

            q_tiles = [None] * S_PER_CORE  # per-sample [q1, q2, q3, q4]

            def emit_layer(s, li, m_range, q_prev_fn):
                cin, cout = LAYERS[li]
                kt = cin // 128
                qn = q_tiles[s][li]
                for m in m_range:
                    ps = psum_pool.tile([128, HW], f32, tag="ps")
                    for k in range(kt):
                        nc.tensor.matmul(
                            ps[:],
                            w_slice(s, li, k, m),
                            q_prev_fn(k),
                            start=(k == 0),
                            stop=(k == kt - 1),
                        )
                    bcol = s * BIAS_COLS + BIAS_COL0[li] + m
                    nc.scalar.activation(
                        qn[:, m * HW : (m + 1) * HW],
                        ps[:],
                        sig,
                        bias=bias_sb[:, bcol : bcol + 1],
                        scale=ACT_SCALE[li],
                    )

            def emit_fc5(s):
                ps5 = psum_pool.tile([128, HW], f32, tag="ps", name=f"ps5_{s}")
                w5t = w5_sb[:, s * W5_COLS : (s + 1) * W5_COLS]
                nc.tensor.matmul(
                    ps5[0:32, :], w5t, q_tiles[s][3][:, 0:HW], start=True, stop=True
                )
                b5col = s * BIAS_COLS + 15
                nc.scalar.activation(
                    ot_all[0:1, s * HW : (s + 1) * HW],
                    ps5[0:1, :],
                    ident,
                    bias=bias_sb[0:1, b5col : b5col + 1],
                    scale=1.0,
                )
                nc.scalar.dma_start(
                    out_d[0:1, s * HW : (s + 1) * HW],
                    ot_all[0:1, s * HW : (s + 1) * HW],
                )

            def alloc_q(s):
                q_tiles[s] = [
                    qpool.tile(
                        [128, (LAYERS[li][1] // 128) * HW],
                        f16,
                        tag=f"q{li}",
                        name=f"q{li}_{s}",
                    )
                    for li in range(4)
                ]

            def qprev(s, li):
                return lambda k, s=s, li=li: q_tiles[s][li][:, k * HW : (k + 1) * HW]

            last = S_PER_CORE - 1
            for s in range(last):
                alloc_q(s)
                xf = lambda k, s=s: x_slice(s, k)
                # fc1 m0-3; the previous sample's fc4 resolves its fc3
                # activations behind these 2.7 us of matmuls
                emit_layer(s, 0, range(0, 4), xf)
                if s > 0:
                    emit_layer(s - 1, 3, range(0, 1), qprev(s - 1, 2))
                emit_layer(s, 0, range(4, 8), xf)
                if s > 0:
                    emit_fc5(s - 1)
                emit_layer(s, 1, range(0, 4), qprev(s, 0))
                emit_layer(s, 2, range(0, 2), qprev(s, 1))
                # the last sample's fc1 m-pairs run in mid-stream idle
                if s == last - 2:
                    alloc_q(last)
                    emit_layer(last, 0, range(0, 2), lambda k: x_slice(last, k))
                if s == last - 1:
                    emit_layer(last, 0, range(2, 4), lambda k: x_slice(last, k))
            # tail: only half of s7's fc1 + its small layers remain, with
            # s6's fc4/fc5 interleaved to hide the ACT edges
            emit_layer(last, 0, range(4, 6), lambda k: x_slice(last, k))
            emit_layer(last - 1, 3, range(0, 1), qprev(last - 1, 2))
            emit_layer(last, 0, range(6, 8), lambda k: x_slice(last, k))
            emit_fc5(last - 1)
            emit_layer(last, 1, range(0, 4), qprev(last, 0))
            emit_layer(last, 2, range(0, 2), qprev(last, 1))
            emit_layer(last, 3, range(0, 1), qprev(last, 2))
            emit_fc5(last)

    _split_ctrl_multiwaits(nc)
    return nc


_NC_CACHE = None


def _get_nc():
    global _NC_CACHE
    if _NC_CACHE is None:
        _NC_CACHE = _build_nc()
    return _NC_CACHE


def _to_e3m4(a, scale):
    return np.clip(a * scale, -14.0, 14.0).astype(ml_dtypes.float8_e3m4)


def _prep_core(inputs, c):
    """Build the per-core input map (numpy only, host-side layout prep)."""
    sl = slice(c * S_PER_CORE, (c + 1) * S_PER_CORE)

    def wimg(li):
        cin, cout = LAYERS[li]
        w = inputs[f"target_fc{li + 1}w"][sl, :, :, 0, 0]  # [S, Cout, Cin]
        # -> [S, 128, (Cin/128)*Cout] with img[s, p, k*Cout+co] = w[s, co, k*128+p]
        wt = w.transpose(0, 2, 1).reshape(S_PER_CORE, cin // 128, 128, cout)
        return wt.transpose(0, 2, 1, 3).reshape(S_PER_CORE, 128, -1)

    w1 = wimg(0)  # [S, 128, 16384] k-major: col = k*1024 + m*128
    # fc1 -> m-major: col = m*2048 + k*128
    w1 = (
        w1.reshape(S_PER_CORE, 128, 16, 8, 128)
        .transpose(0, 1, 3, 2, 4)
        .reshape(S_PER_CORE, 128, A_COLS)
    )
    wslab = np.ascontiguousarray(
        _to_e3m4(np.concatenate([w1] + [wimg(li) for li in (1, 2, 3)], axis=2), W_SCALE_FP8)
    ).view(np.uint8).reshape(S_PER_CORE, 128, -1).view(np.float16)

    x = inputs["target_in_vec"][sl].reshape(S_PER_CORE, 2048 // 128, 128, HW)
    ximg = x.transpose(2, 0, 1, 3).reshape(128, S_PER_CORE * X_COLS)
    ximg = np.ascontiguousarray(_to_e3m4(ximg, X_SCALE_FP8)).view(np.uint8).view(np.float16)

    w5 = inputs["target_fc5w"][sl, 0, :, 0, 0].astype(np.float16)  # [S, 128]
    w5img = np.zeros((128, S_PER_CORE, W5_COLS), np.float16)
    w5img[:, :, 0] = w5.T
    w5img = np.ascontiguousarray(w5img.reshape(128, -1))

    bias = np.zeros((S_PER_CORE, 128, BIAS_COLS), np.float32)
    for li, (cin, cout) in enumerate(LAYERS):
        b = inputs[f"target_fc{li + 1}b"][sl]  # [S, Cout]
        bias[:, :, BIAS_COL0[li] : BIAS_COL0[li] + cout // 128] = b.reshape(
            S_PER_CORE, cout // 128, 128
        ).transpose(0, 2, 1)
    bias[:, 0, 15] = inputs["target_fc5b"][sl, 0]
    bias = np.ascontiguousarray(bias.transpose(1, 0, 2).reshape(128, -1))

    return {"wslab": wslab, "ximg": ximg, "w5img": w5img, "bias": bias}


def kernel(**inputs):
    inputs = {k: np.asarray(v) for k, v in inputs.items()}
    nc = _get_nc()
    in_maps = [_prep_core(inputs, c) for c in range(N_CORES)]
    res = run_bass_kernel_spmd(nc, in_maps, list(range(N_CORES)))
    out = np.concatenate([np.asarray(res.results[c]["out"]) for c in range(N_CORES)], axis=0)
    return out.reshape(B, 8, 8).astype(np.float32)


# revision 38
# speedup vs baseline: 1.0880x; 1.0880x over previous
"""Trainium2 Bass kernel for nn_BaselineTargetHead (per-sample dynamic MLP).

Strategy: data-parallel over 8 NeuronCores, 8 samples per core.
Per sample the chain is 5 per-sample linear layers over 64 spatial positions:
  [1024,2048] @ [2048,64] -> sigmoid -> ... -> [1,128] @ [128,64] + b

fc1-fc4 weights (99.9% of bytes) and the input x ship as fp8 e3m4 (4
mantissa bits). Host pre-scales weights by 64 (x by 2) to center N(0,0.02)
data in e3m4's normal range; the inverse scale folds into the ScalarE
activation's `scale`. fc5 weights stay fp16: the output is a 128-term dot
product with no downstream averaging, so fc5 quantization dominates the
error budget (quantizing w5 alone costs 1.4e-2 rel err; w1-w4 cost ~1e-3).

The kernel sits at the ridge: Tensor ~55-61 us busy (40-47 ns per
LDWEIGHTS+MATMUL pair, 171 pairs/sample) vs DMA ~68 us on a single queue.
The 8 cores contend for chip HBM (~2.6-2.9 TB/s aggregate), so per-core
stream rate tops out around 330-350 B/ns; splitting across two HWDGE
queues (or gpsimd SWDGE) derates the shared fabric and loses. Scheduling
details that matter:
  - everything lives in SBUF simultaneously (~186 KB/partition), so all
    DMAs are issued upfront with no tile rotation. Bulk DMAs must NOT be
    issued from the ACT engine: the tile framework's per-queue flow
    control would block ScalarE (and all activations) behind transfers.
  - weight DMAs are typed fp16 and bitcast to fp8 at the matmul.
  - fc1 is laid out m-major (col = m*2048 + k*128) and shipped in chunks
    that enable complete m-tiles immediately; sample 0's fc1 arrives in
    four chunks so the PE starts early.
  - the LAST sample's fc1 m0-3 chunks are spread early in the stream
    (after s4/s5) and their matmuls emitted inside blocks s5/s6, filling
    the mid-stream DMA-paced PE idle; only half its fc1 plus fc2-5
    remains after the final stream byte.
  - the previous sample's tiny fc4/fc5 are interleaved into fc1's m-groups
    so their input activations (314 ns ScalarE latency each) resolve
    behind ~2.7 us of fc1 matmuls instead of stalling the PE (~1.4
    us/sample of layer-boundary gaps otherwise).
  - matmul: lhsT = W^T tile [128(Cin), 128(Cout)] fp8 (FWL halves the
    weight-load time), rhs = activation tile [128(Cin), 64(spatial)] fp16,
    accumulated over Cin tiles in PSUM fp32. ScalarE applies
    scale+bias+sigmoid fused, writing fp16 tiles that feed the next layer
    without any transposition.
"""

import numpy as np
import ml_dtypes

import concourse.bass as bass
import concourse.mybir as mybir
import concourse.tile as tile
from concourse.bass_utils import run_bass_kernel_spmd

N_CORES = 8
B = 64
S_PER_CORE = B // N_CORES  # 8 samples per core
HW = 64  # 8x8 spatial positions
LAYERS = [(2048, 1024), (1024, 512), (512, 256), (256, 128)]  # (Cin, Cout) of fc1..fc4
W_SCALE_FP8 = 64.0  # host multiplies fp8 weights by this; kernel divides back
X_SCALE_FP8 = 2.0  # same for the input x image
A_COLS = (LAYERS[0][0] // 128) * LAYERS[0][1]  # 16384 fp8 cols (fc1, m-major)
B_COLS = sum((ci // 128) * co for ci, co in LAYERS[1:])  # 5376 fp8 cols (fc2-4)
X_COLS = (2048 // 128) * HW  # 1024
W5_COLS = 32  # w5 zero-padded to 32 cols for a legal M=32 matmul
# bias image columns per sample: fc1 m0..7 | fc2 m0..3 | fc3 m0..1 | fc4 m0 | fc5
BIAS_COL0 = [0, 8, 12, 14]
BIAS_COLS = 16
# per-layer PSUM scale to undo the host-side fp8 pre-scaling
ACT_SCALE = [
    1.0 / (W_SCALE_FP8 * X_SCALE_FP8),
    1.0 / W_SCALE_FP8,
    1.0 / W_SCALE_FP8,
    1.0 / W_SCALE_FP8,
]
# per-layer base fp8 column of each layer's weights within the fc2-4 block
B_OFF = [0, 4096, 4096 + 1024]  # fc2, fc3, fc4


def _split_ctrl_multiwaits(nc):
    """walrus in this env rejects >1 sync-wait per instruction. Move extra
    waits onto NOPs placed immediately before, on the same engine — engines
    execute in order, so this is semantically identical."""
    n_fixed = 0
    for bb in nc.main_func.blocks:
        insts = bb.instructions
        i = 0
        while i < len(insts):
            ins = insts[i]
            si = ins.sync_info
            if si is not None and si.on_wait and len(si.on_wait) > 1:
                waits = list(si.on_wait)
                new_nops = []
                for j, w in enumerate(waits[1:]):
                    nop = mybir.InstNoOp(name=f"{ins.name}-splitw-{j}", ins=[], outs=[])
                    nop.engine = ins.engine
                    nop.sync_info = mybir.SyncInfo(on_update=[], on_wait=[w])
                    new_nops.append(nop)
                si.on_wait = [waits[0]]
                insts[i:i] = new_nops
                i += len(new_nops)
                n_fixed += 1
            i += 1
    return n_fixed


def _build_nc():
    f8 = mybir.dt.float8e3
    f16 = mybir.dt.float16
    f32 = mybir.dt.float32
    nc = bass.Bass()
    # weight/x images carry fp8 bytes but are typed fp16 for the DMA;
    # compute slices bitcast back to fp8.
    wslab_d = nc.dram_tensor(
        "wslab", [S_PER_CORE, 128, (A_COLS + B_COLS) // 2], f16, kind="ExternalInput"
    )
    ximg_d = nc.dram_tensor(
        "ximg", [128, S_PER_CORE * X_COLS // 2], f16, kind="ExternalInput"
    )
    w5img_d = nc.dram_tensor("w5img", [128, S_PER_CORE * W5_COLS], f16, kind="ExternalInput")
    bias_d = nc.dram_tensor("bias", [128, S_PER_CORE * BIAS_COLS], f32, kind="ExternalInput")
    out_d = nc.dram_tensor("out", [1, S_PER_CORE * HW], f32, kind="ExternalOutput")

    sig = mybir.ActivationFunctionType.Sigmoid
    ident = mybir.ActivationFunctionType.Identity

    with tile.TileContext(nc) as tc:
        with (
            tc.tile_pool(name="wpool", bufs=1) as wpool,
            tc.tile_pool(name="qpool", bufs=2) as qpool,
            tc.tile_pool(name="psum", bufs=6, space="PSUM") as psum_pool,
        ):
            # ---- all DMAs issued upfront; everything fits in SBUF ----
            # SP queue: sample 0's x slice first (it gates the first matmul),
            # then the weight slabs, sample-major: fc1 m0-3 | fc1 m4-7 | fc2-4
            # (sample 0's fc1 in four chunks so the PE starts sooner).
            x_sb = wpool.tile([128, S_PER_CORE * X_COLS // 2], f16)
            nc.sync.dma_start(x_sb[:, 0 : X_COLS // 2], ximg_d[:, 0 : X_COLS // 2])
            wa_sb, wb_sb = [], []
            for s in range(S_PER_CORE):
                ta = wpool.tile([128, A_COLS // 2], f16, name=f"wa{s}")
                wa_sb.append(ta)
                tb = wpool.tile([128, B_COLS // 2], f16, name=f"wb{s}")
                wb_sb.append(tb)
            AH = A_COLS // 2  # 8192 f16 cols of fc1 per sample
            QP = AH // 4  # 2048 f16 cols per m-pair (m-major layout)

            def issue_a(s, a, b):
                nc.sync.dma_start(wa_sb[s][:, a:b], wslab_d[s, :, a:b])

            def issue_b(s):
                nc.sync.dma_start(
                    wb_sb[s][:], wslab_d[s, :, AH : AH + B_COLS // 2]
                )

            LS = S_PER_CORE - 1
            # Sample-major stream, but the LAST sample's fc1 m-pairs are
            # spread early (after s4/s5) so its matmuls can run in the
            # mid-stream DMA-paced idle and the serial tail shrinks to
            # ~half of fc1 + fc2-5.
            for s in range(LS):
                nchunk = 4 if s == 0 else 2
                step = AH // nchunk
                for j in range(nchunk):
                    issue_a(s, j * step, (j + 1) * step)
                issue_b(s)
                if s == LS - 3:
                    issue_a(LS, 0, QP)  # s7 fc1 m0-1
                if s == LS - 2:
                    issue_a(LS, QP, 2 * QP)  # s7 fc1 m2-3
            issue_a(LS, 2 * QP, 3 * QP)  # s7 fc1 m4-5
            issue_a(LS, 3 * QP, 4 * QP)  # s7 fc1 m6-7
            issue_b(LS)
            # ACT queue: rest of x, bias, w5 — small transfers ride the
            # second queue essentially for free; it drains early so the
            # per-sample output DMAs are never stuck behind weight traffic.
            nc.scalar.dma_start(x_sb[:, X_COLS // 2 :], ximg_d[:, X_COLS // 2 :])
            bias_sb = wpool.tile([128, S_PER_CORE * BIAS_COLS], f32)
            nc.scalar.dma_start(bias_sb[:], bias_d[:])
            w5_sb = wpool.tile([128, S_PER_CORE * W5_COLS], f16)
            nc.scalar.dma_start(w5_sb[:], w5img_d[:])
            # all samples' outputs land in partition 0 of one tile
            # (sample s -> columns s*HW..(s+1)*HW)
            ot_all = wpool.tile([128, S_PER_CORE * HW], f32)

            # ---- compute ----
            def w_slice(s, li, k, m):
                """fp8 lhsT [128, 128] for (layer, k-tile, m-tile) of sample s."""
                if li == 0:
                    col = m * 2048 + k * 128  # m-major fc1 layout
                    return wa_sb[s][:, col // 2 : col // 2 + 64].bitcast(
                        mybir.dt.float8e3
                    )
                col = B_OFF[li - 1] + k * LAYERS[li][1] + m * 128
                return wb_sb[s][:, col // 2 : col // 2 + 64].bitcast(mybir.dt.float8e3)

            def x_slice(s, k):
                c = s * X_COLS + k * HW
                return x_sb[:, c // 2 : c // 2 + HW // 2].bitcast(mybir.dt.float8e3)

            q_tiles = [None] * S_PER_CORE  # per-sample [q1, q2, q3, q4]

            def emit_layer(s, li, m_range, q_prev_fn):
                cin, cout = LAYERS[li]
                kt = cin // 128
                qn = q_tiles[s][li]
                for m in m_range:
                    ps = psum_pool.tile([128, HW], f32, tag="ps")
                    for k in range(kt):
                        nc.tensor.matmul(
                            ps[:],
                            w_slice(s, li, k, m),
                            q_prev_fn(k),
                            start=(k == 0),
                            stop=(k == kt - 1),
                        )
                    bcol = s * BIAS_COLS + BIAS_COL0[li] + m
                    nc.scalar.activation(
                        qn[:, m * HW : (m + 1) * HW],
                        ps[:],
                        sig,
                        bias=bias_sb[:, bcol : bcol + 1],
                        scale=ACT_SCALE[li],
                    )

            def emit_fc5(s):
                ps5 = psum_pool.tile([128, HW], f32, tag="ps", name=f"ps5_{s}")
                w5t = w5_sb[:, s * W5_COLS : (s + 1) * W5_COLS]
                nc.tensor.matmul(
                    ps5[0:32, :], w5t, q_tiles[s][3][:, 0:HW], start=True, stop=True
                )
                b5col = s * BIAS_COLS + 15
                nc.scalar.activation(
                    ot_all[0:1, s * HW : (s + 1) * HW],
                    ps5[0:1, :],
                    ident,
                    bias=bias_sb[0:1, b5col : b5col + 1],
                    scale=1.0,
                )
                nc.scalar.dma_start(
                    out_d[0:1, s * HW : (s + 1) * HW],
                    ot_all[0:1, s * HW : (s + 1) * HW],
                )

            def alloc_q(s):
                q_tiles[s] = [
                    qpool.tile(
                        [128, (LAYERS[li][1] // 128) * HW],
                        f16,
                        tag=f"q{li}",
                        name=f"q{li}_{s}",
                    )
                    for li in range(4)
                ]

            def qprev(s, li):
                return lambda k, s=s, li=li: q_tiles[s][li][:, k * HW : (k + 1) * HW]

            last = S_PER_CORE - 1
            for s in range(last):
                alloc_q(s)
                xf = lambda k, s=s: x_slice(s, k)
                # fc1 m0-3; the previous sample's fc4 resolves its fc3
                # activations behind these 2.7 us of matmuls
                emit_layer(s, 0, range(0, 4), xf)
                if s > 0:
                    emit_layer(s - 1, 3, range(0, 1), qprev(s - 1, 2))
                emit_layer(s, 0, range(4, 8), xf)
                if s > 0:
                    emit_fc5(s - 1)
                emit_layer(s, 1, range(0, 4), qprev(s, 0))
                emit_layer(s, 2, range(0, 2), qprev(s, 1))
                # the last sample's fc1 m-pairs run in mid-stream idle
                if s == last - 2:
                    alloc_q(last)
                    emit_layer(last, 0, range(0, 2), lambda k: x_slice(last, k))
                if s == last - 1:
                    emit_layer(last, 0, range(2, 4), lambda k: x_slice(last, k))
            # tail: only half of s7's fc1 + its small layers remain, with
            # s6's fc4/fc5 interleaved to hide the ACT edges
            emit_layer(last, 0, range(4, 6), lambda k: x_slice(last, k))
            emit_layer(last - 1, 3, range(0, 1), qprev(last - 1, 2))
            emit_layer(last, 0, range(6, 8), lambda k: x_slice(last, k))
            emit_fc5(last - 1)
            emit_layer(last, 1, range(0, 4), qprev(last, 0))
            emit_layer(last, 2, range(0, 2), qprev(last, 1))
            emit_layer(last, 3, range(0, 1), qprev(last, 2))
            emit_fc5(last)

    _split_ctrl_multiwaits(nc)
    return nc


_NC_CACHE = None


def _get_nc():
    global _NC_CACHE
    if _NC_CACHE is None:
        _NC_CACHE = _build_nc()
    return _NC_CACHE


def _to_e3m4(a, scale):
    return np.clip(a * scale, -14.0, 14.0).astype(ml_dtypes.float8_e3m4)


def _prep_core(inputs, c):
    """Build the per-core input map (numpy only, host-side layout prep)."""
    sl = slice(c * S_PER_CORE, (c + 1) * S_PER_CORE)

    def wimg(li):
        cin, cout = LAYERS[li]
        w = inputs[f"target_fc{li + 1}w"][sl, :, :, 0, 0]  # [S, Cout, Cin]
        # -> [S, 128, (Cin/128)*Cout] with img[s, p, k*Cout+co] = w[s, co, k*128+p]
        wt = w.transpose(0, 2, 1).reshape(S_PER_CORE, cin // 128, 128, cout)
        return wt.transpose(0, 2, 1, 3).reshape(S_PER_CORE, 128, -1)

    w1 = wimg(0)  # [S, 128, 16384] k-major: col = k*1024 + m*128
    # fc1 -> m-major: col = m*2048 + k*128
    w1 = (
        w1.reshape(S_PER_CORE, 128, 16, 8, 128)
        .transpose(0, 1, 3, 2, 4)
        .reshape(S_PER_CORE, 128, A_COLS)
    )
    wslab = np.ascontiguousarray(
        _to_e3m4(np.concatenate([w1] + [wimg(li) for li in (1, 2, 3)], axis=2), W_SCALE_FP8)
    ).view(np.uint8).reshape(S_PER_CORE, 128, -1).view(np.float16)

    x = inputs["target_in_vec"][sl].reshape(S_PER_CORE, 2048 // 128, 128, HW)
    ximg = x.transpose(2, 0, 1, 3).reshape(128, S_PER_CORE * X_COLS)
    ximg = np.ascontiguousarray(_to_e3m4(ximg, X_SCALE_FP8)).view(np.uint8).view(np.float16)

    w5 = inputs["target_fc5w"][sl, 0, :, 0, 0].astype(np.float16)  # [S, 128]
    w5img = np.zeros((128, S_PER_CORE, W5_COLS), np.float16)
    w5img[:, :, 0] = w5.T
    w5img = np.ascontiguousarray(w5img.reshape(128, -1))

    bias = np.zeros((S_PER_CORE, 128, BIAS_COLS), np.float32)
    for li, (cin, cout) in enumerate(LAYERS):
        b = inputs[f"target_fc{li + 1}b"][sl]  # [S, Cout]
        bias[:, :, BIAS_COL0[li] : BIAS_COL0[li] + cout // 128] = b.reshape(
            S_PER_CORE, cout // 128, 128
        ).transpose(0, 2, 1)
    bias[:, 0, 15] = inputs["target_fc5b"][sl, 0]
    bias = np.ascontiguousarray(bias.transpose(1, 0, 2).reshape(128, -1))

    return {"wslab": wslab, "ximg": ximg, "w5img": w5img, "bias": bias}


def kernel(**inputs):
    inputs = {k: np.asarray(v) for k, v in inputs.items()}
    nc = _get_nc()
    in_maps = [_prep_core(inputs, c) for c in range(N_CORES)]
    res = run_bass_kernel_spmd(nc, in_maps, list(range(N_CORES)))
    out = np.concatenate([np.asarray(res.results[c]["out"]) for c in range(N_CORES)], axis=0)
    return out.reshape(B, 8, 8).astype(np.float32)
